# revision 16
# baseline (speedup 1.0000x reference)
"""ClusterAttention (segment_reduce) Trainium2 kernel, v2 (bf16).

Strategy: shard by cluster ("bucket"). The host groups point indices by
bucket (argsort of cluster_ids — pure index preprocessing), pads each
bucket's point list to a multiple of 16 (a "slot") by duplicating the
bucket's first point, and cuts the bucket list into 8 contiguous,
point-balanced core shards. Every core owns whole buckets — NO
cross-core communication. All large tensors move as bf16 (x is cast on
host); matmuls run bf16 on the PE with fp32 PSUM accumulation.

  pass 1: xT chunks (host-pretransposed bf16) -> k^T,v^T matmuls; DVE
          slot maxes of k^T,v^T (interleaved d=2 for gather) and slot
          sums of x (f32, includes pad duplicates); Pool saves each
          slot's first element (for the pad-sum correction).
  mid:    slot->bucket combine (gpsimd ap_gather + DVE reduce). Bucket
          x-sums are corrected by npad_b * x_first_b (pads duplicate the
          bucket's first point, whose value is firsts[sstart_b]). Then,
          using segment-sum linearity (k_sum[b] = x_sum[b] @ Wk + cnt*bk),
          build the per-bucket context table (d=4 bf16 for gather):
            tbl[...,0] = scale*(x_mean@Wk + bk)        (k_mean, pre-scaled)
            tbl[...,1] = scale*(max_k + bk)            (k_max,  pre-scaled)
            tbl[...,2] = [v_mean|v_max] @ Wvc + bvc    (v_combined)
          and expand to per-slot context (ap_gather).
  pass 2: re-read xT chunks -> q^T; one fused DVE op forms both
          interaction halves ((q^T+bq)*ctx01); gate MLP (PE + Act
          drains); gv on Pool; out^T = Wp.T @ (gate*v_comb) + bp.

The output is returned transposed+permuted bf16; the host scatters rows
back (duplicate pad rows rewrite identical values).
"""

import numpy as np
import ml_dtypes

import concourse.bass as bass
from concourse import bacc
import concourse.mybir as mybir
import concourse.tile as tile
import concourse.bass_utils as bass_utils

# problem constants (hardcoded per contract)
N_GLOBAL = 500000
C = 128
B_GLOBAL = 8192
NUM_HEADS = 4
NCORES = 8
SLOT = 16          # points per slot (pad unit)
CHUNK = 512        # points per device chunk
SCALE = float((C // NUM_HEADS) ** (-0.5))
NEG_BIG = -1.0e30

f32 = mybir.dt.float32
bf16 = mybir.dt.bfloat16
i16 = mybir.dt.int16
np_bf16 = ml_dtypes.bfloat16
X = mybir.AxisListType.X
ALU = mybir.AluOpType
ACTF = mybir.ActivationFunctionType


def _wrap16(vals):
    """ap_gather index layout: [128, n//16] int16, idx j read from
    partition j%16 (replicated across the 8 gpsimd cores' groups)."""
    v = np.asarray(vals, np.int16)
    n = v.size
    assert n % 16 == 0
    arr = np.zeros((128, n // 16), np.int16)
    k = np.arange(n)
    for g in range(8):
        arr[16 * g + (k % 16), k // 16] = v
    return arr


def _build_layout(ids, B, ncores):
    """Bucket-grouped, slot-padded permutation + all per-core metadata."""
    N = ids.shape[0]
    counts = np.bincount(ids, minlength=B).astype(np.int64)
    order = np.argsort(ids, kind="stable")
    starts = np.zeros(B + 1, np.int64)
    starts[1:] = np.cumsum(counts)
    nslots_b = (counts + SLOT - 1) // SLOT
    padded_b = nslots_b * SLOT

    cum = np.cumsum(padded_b)
    total = cum[-1]
    cuts = [0]
    for c in range(1, ncores):
        cuts.append(int(np.searchsorted(cum, c * total / ncores)))
    cuts.append(B)

    core_npts = [int(padded_b[cuts[c]:cuts[c + 1]].sum()) for c in range(ncores)]
    core_nbux = [cuts[c + 1] - cuts[c] for c in range(ncores)]
    NPTS = max(2 * CHUNK, int(-(-max(core_npts) // (2 * CHUNK))) * 2 * CHUNK)
    NBUX = max(16, int(-(-max(core_nbux) // 16)) * 16)
    NSLOT = NPTS // SLOT
    SPB = max(1, int(nslots_b.max()))  # max slots per bucket

    cores = []
    for c in range(ncores):
        bs, be = cuts[c], cuts[c + 1]
        nb = be - bs
        cnts = counts[bs:be]
        nsl = nslots_b[bs:be]
        sstart = np.zeros(nb + 1, np.int64)
        sstart[1:] = np.cumsum(nsl)
        tot_slots = int(sstart[-1])
        dst0 = sstart[:-1] * SLOT

        perm = np.full(NPTS, -1, np.int64)
        src = order[starts[bs]:starts[be]]
        if src.size:
            seg0 = (starts[bs:be] - starts[bs])
            pos = np.repeat(dst0, cnts) + (np.arange(src.size) - np.repeat(seg0, cnts))
            perm[pos] = src
        is_real = perm >= 0
        padcnt = (nsl * SLOT - cnts)
        if padcnt.sum():
            off = np.concatenate([[0], np.cumsum(padcnt)])[:-1]
            ppos = np.repeat(dst0 + cnts, padcnt) + (np.arange(int(padcnt.sum())) - np.repeat(off, padcnt))
            first = np.where(cnts > 0, order[starts[bs:be]], 0)
            perm[ppos] = np.repeat(first, padcnt)

        cnt_rep = np.ones(NBUX, np.float32)
        cnt_rep[:nb] = np.maximum(cnts, 1).astype(np.float32)
        cnt_rep = np.broadcast_to(cnt_rep, (128, NBUX)).copy()
        npad_rep = np.zeros(NBUX, np.float32)
        npad_rep[:nb] = padcnt.astype(np.float32)
        npad_rep = np.broadcast_to(npad_rep, (128, NBUX)).copy()

        jj = np.arange(SPB)[None, :]
        bmat = np.where(jj < nsl[:, None], sstart[:-1][:, None] + jj, NSLOT)
        L = np.full(NBUX * SPB, NSLOT, np.int64)
        L[: nb * SPB] = bmat.ravel()
        bidx = _wrap16(L)

        S = np.full(NSLOT, NBUX, np.int64)
        if tot_slots:
            S[:tot_slots] = np.repeat(np.arange(nb), nsl)
        sidx = _wrap16(S)

        F = np.full(NBUX, NSLOT, np.int64)
        F[:nb] = sstart[:-1]
        fidx = _wrap16(F)

        cores.append(dict(perm=perm, is_real=is_real, cnt=cnt_rep,
                          npad=npad_rep, bidx=bidx, sidx=sidx, fidx=fidx))
    return dict(NPTS=NPTS, NBUX=NBUX, NSLOT=NSLOT, SPB=SPB,
                NCHUNK=NPTS // CHUNK, cores=cores)


def _build_nc(L, reps=1):
    NPTS, NBUX, NSLOT, SPB = L["NPTS"], L["NBUX"], L["NSLOT"], L["SPB"]
    NCHUNK = L["NCHUNK"]

    nc = bacc.Bacc("TRN2", target_bir_lowering=False)
    xtp = nc.dram_tensor("xtp", [C, NPTS], bf16, kind="ExternalInput")
    cnt_d = nc.dram_tensor("cnt", [128, NBUX], f32, kind="ExternalInput")
    npad_d = nc.dram_tensor("npad", [128, NBUX], f32, kind="ExternalInput")
    bidx_d = nc.dram_tensor("bidx", [128, (NBUX * SPB) // 16], i16, kind="ExternalInput")
    sidx_d = nc.dram_tensor("sidx", [128, NSLOT // 16], i16, kind="ExternalInput")
    fidx_d = nc.dram_tensor("fidx", [128, NBUX // 16], i16, kind="ExternalInput")
    wb_d = {}
    for nm in ["Wq", "Wk", "Wv", "Wg1a", "Wg1b", "Wg2", "Wp"]:
        wb_d[nm] = nc.dram_tensor(nm + "h", [C, C], bf16, kind="ExternalInput")
    wf_d = {}
    for nm in ["Wk", "Wv", "Wvca", "Wvcb"]:
        wf_d[nm] = nc.dram_tensor(nm + "f", [C, C], f32, kind="ExternalInput")
    b_d = {}
    for nm in ["bq", "bk", "bv", "bg1", "bg2", "bvc", "bp"]:
        b_d[nm] = nc.dram_tensor(nm, [C], f32, kind="ExternalInput")
    ot = nc.dram_tensor("ot", [C, NPTS], bf16, kind="ExternalOutput")

    def _emit(tc):
        with tc.tile_pool(name="consts", bufs=1) as consts, \
             tc.tile_pool(name="tables", bufs=1) as tables:
            wb = {}
            for nm in wb_d:
                wb[nm] = consts.tile([C, C], bf16, name="wb_" + nm, tag="wb_" + nm)
                nc.sync.dma_start(out=wb[nm][:], in_=wb_d[nm][:])
            wf = {}
            for nm in wf_d:
                wf[nm] = consts.tile([C, C], f32, name="wf_" + nm, tag="wf_" + nm)
                nc.sync.dma_start(out=wf[nm][:], in_=wf_d[nm][:])
            b = {}
            for nm in b_d:
                b[nm] = consts.tile([C, 1], f32, name="b_" + nm, tag="b_" + nm)
                nc.sync.dma_start(out=b[nm][:], in_=b_d[nm][:, None])
            bk_s = consts.tile([C, 1], f32)
            nc.scalar.activation(out=bk_s[:], in_=b["bk"][:], func=ACTF.Identity,
                                 scale=SCALE)

            # ---------------- pass 1 ----------------
            with tc.tile_pool(name="slots", bufs=1) as slots, \
                 tc.tile_pool(name="ctx", bufs=1) as ctxp:
                xslot = slots.tile([128, NSLOT + 16], f32)
                kvslot = slots.tile([128, NSLOT + 16, 2], bf16)
                firsts = slots.tile([128, NSLOT + 16], f32)
                nc.vector.memset(xslot[:, NSLOT:], 0.0)
                nc.vector.memset(kvslot[:, NSLOT:, :], NEG_BIG)
                nc.vector.memset(firsts[:, NSLOT:], 0.0)

                with tc.tile_pool(name="p1w", bufs=3) as p1w, \
                     tc.tile_pool(name="p1ps", bufs=2, space="PSUM") as p1ps:
                    def ev(t):  # even/odd element views along last axis
                        r = t.rearrange("p s (j two) -> p s j two", two=2)
                        return r[:, :, :, 0], r[:, :, :, 1]

                    for cj in range(NCHUNK // 2):
                        s0 = cj * 64  # slots per chunk-pair
                        xt2 = p1w.tile([128, 2 * CHUNK], bf16, tag="xt2")
                        nc.sync.dma_start(
                            out=xt2[:],
                            in_=xtp[:, cj * 2 * CHUNK:(cj + 1) * 2 * CHUNK])
                        kv_ps = p1ps.tile([128, 2, 2 * CHUNK], f32, tag="kv")
                        for hh in range(2):
                            sl = slice(hh * CHUNK, (hh + 1) * CHUNK)
                            nc.tensor.matmul(out=kv_ps[:, 0, sl], lhsT=wb["Wk"][:],
                                             rhs=xt2[:, sl], start=True, stop=True)
                            nc.tensor.matmul(out=kv_ps[:, 1, sl], lhsT=wb["Wv"][:],
                                             rhs=xt2[:, sl], start=True, stop=True)
                        # drain kv PSUM -> SBUF bf16 (Act), then pairwise
                        # max trees 16->1: r1-r2 DVE (2x all-SBUF), r3-r4 Pool
                        kvsb = p1w.tile([128, 2, 2 * CHUNK], bf16, tag="kvsb")
                        nc.scalar.copy(out=kvsb[:], in_=kv_ps[:])
                        a, bb = ev(kvsb)
                        kv1 = p1w.tile([128, 2, CHUNK], bf16, tag="kv1")
                        nc.vector.scalar_tensor_tensor(
                            out=kv1[:], in0=a, scalar=0.0, in1=bb,
                            op0=ALU.add, op1=ALU.max)
                        a, bb = ev(kv1)
                        kv2 = p1w.tile([128, 2, CHUNK // 2], bf16, tag="kv2")
                        nc.vector.scalar_tensor_tensor(
                            out=kv2[:], in0=a, scalar=0.0, in1=bb,
                            op0=ALU.add, op1=ALU.max)
                        a, bb = ev(kv2)
                        kv3 = p1w.tile([128, 2, CHUNK // 4], bf16, tag="kv3")
                        nc.vector.scalar_tensor_tensor(
                            out=kv3[:], in0=a, scalar=0.0, in1=bb,
                            op0=ALU.add, op1=ALU.max)
                        a, bb = ev(kv3)
                        nc.vector.scalar_tensor_tensor(
                            out=kvslot[:, s0:s0 + 64, :].rearrange("p s u -> p u s"),
                            in0=a, scalar=0.0, in1=bb, op0=ALU.add, op1=ALU.max)
                        # x slot sums: r1 DVE (2x), r2-r4 Pool, f32 last round
                        xtr = xt2[:].rearrange("p (s two) -> p s two", two=2)
                        x1 = p1w.tile([128, CHUNK], bf16, tag="x1")
                        nc.vector.scalar_tensor_tensor(
                            out=x1[:], in0=xtr[:, :, 0], scalar=0.0,
                            in1=xtr[:, :, 1], op0=ALU.add, op1=ALU.add)
                        x1r = x1[:].rearrange("p (s two) -> p s two", two=2)
                        x2 = p1w.tile([128, CHUNK // 2], bf16, tag="x2")
                        nc.gpsimd.tensor_tensor(out=x2[:], in0=x1r[:, :, 0],
                                                in1=x1r[:, :, 1], op=ALU.add)
                        x2r = x2[:].rearrange("p (s two) -> p s two", two=2)
                        x3 = p1w.tile([128, CHUNK // 4], bf16, tag="x3")
                        nc.gpsimd.tensor_tensor(out=x3[:], in0=x2r[:, :, 0],
                                                in1=x2r[:, :, 1], op=ALU.add)
                        x3r = x3[:].rearrange("p (s two) -> p s two", two=2)
                        nc.gpsimd.tensor_tensor(out=xslot[:, s0:s0 + 64],
                                                in0=x3r[:, :, 0], in1=x3r[:, :, 1],
                                                op=ALU.add)
                        # first element of each slot (for pad correction; Act)
                        nc.scalar.copy(
                            out=firsts[:, s0:s0 + 64],
                            in_=xt2[:].rearrange("p (s e) -> p s e", e=SLOT)[:, :, 0])

                # ---------------- mid phase ----------------
                tbl = ctxp.tile([128, NBUX + 16, 4], bf16)
                ctxslot = tables.tile([128, NSLOT + 16, 4], bf16)
                with tc.tile_pool(name="midw", bufs=1) as midw, \
                     tc.tile_pool(name="midps", bufs=2, space="PSUM") as midps:
                    nc.vector.memset(tbl[:], 0.0)
                    bidx_sb = midw.tile([128, (NBUX * SPB) // 16], i16, tag="bidx")
                    nc.sync.dma_start(out=bidx_sb[:], in_=bidx_d[:])
                    sidx_sb = midw.tile([128, NSLOT // 16], i16, tag="sidx")
                    nc.sync.dma_start(out=sidx_sb[:], in_=sidx_d[:])
                    fidx_sb = midw.tile([128, NBUX // 16], i16, tag="fidx")
                    nc.sync.dma_start(out=fidx_sb[:], in_=fidx_d[:])
                    cnt_sb = midw.tile([128, NBUX], f32, tag="cnt")
                    nc.sync.dma_start(out=cnt_sb[:], in_=cnt_d[:])
                    npad_sb = midw.tile([128, NBUX], f32, tag="npad")
                    nc.sync.dma_start(out=npad_sb[:], in_=npad_d[:])
                    rc = midw.tile([128, NBUX], f32, tag="rc")
                    nc.vector.reciprocal(out=rc[:], in_=cnt_sb[:])

                    BKB = 512  # buckets per gather block
                    xbsum = midw.tile([128, NBUX], f32, tag="xbsum")
                    kvbmax = midw.tile([128, NBUX, 2], bf16, tag="kvbmax")
                    for j in range(0, NBUX, BKB):
                        e = min(j + BKB, NBUX)
                        nbk = e - j
                        g = midw.tile([128, BKB * SPB], f32, tag="gsx", bufs=2)
                        nc.gpsimd.ap_gather(
                            out_ap=g[:, :nbk * SPB], in_ap=xslot[:, :, None],
                            idxs_ap=bidx_sb[:, (j * SPB) // 16:(e * SPB) // 16],
                            channels=128,
                            num_elems=NSLOT + 16, d=1, num_idxs=nbk * SPB)
                        nc.vector.tensor_reduce(
                            out=xbsum[:, j:e],
                            in_=g[:, :nbk * SPB].rearrange("p (b j) -> p b j", j=SPB),
                            axis=X, op=ALU.add)
                        g2 = midw.tile([128, BKB * SPB, 2], bf16, tag="gskv", bufs=2)
                        nc.gpsimd.ap_gather(
                            out_ap=g2[:, :nbk * SPB, :], in_ap=kvslot[:],
                            idxs_ap=bidx_sb[:, (j * SPB) // 16:(e * SPB) // 16],
                            channels=128,
                            num_elems=NSLOT + 16, d=2, num_idxs=nbk * SPB)
                        nc.vector.tensor_reduce(
                            out=kvbmax[:, j:e, :],
                            in_=g2[:, :nbk * SPB, :].rearrange(
                                "p (b j) u -> p b u j", j=SPB),
                            axis=X, op=ALU.max)

                    fb = midw.tile([128, NBUX], f32, tag="fb")
                    nc.gpsimd.ap_gather(
                        out_ap=fb[:, :, None], in_ap=firsts[:, :, None],
                        idxs_ap=fidx_sb[:], channels=128,
                        num_elems=NSLOT + 16, d=1, num_idxs=NBUX)
                    # x_sum_true = x_sum_dup - npad * x_first; then mean
                    corr = midw.tile([128, NBUX], f32, tag="corr")
                    nc.vector.tensor_tensor(out=corr[:], in0=fb[:], in1=npad_sb[:],
                                            op=ALU.mult)
                    xmean = midw.tile([128, NBUX], f32, tag="xmean")
                    nc.vector.tensor_tensor(out=xmean[:], in0=xbsum[:], in1=corr[:],
                                            op=ALU.subtract)
                    nc.vector.tensor_tensor(out=xmean[:], in0=xmean[:], in1=rc[:],
                                            op=ALU.mult)

                    def mm_big(ps, lhsT, rhs_t, acc=False):
                        for j in range(0, NBUX, 512):
                            e = min(j + 512, NBUX)
                            nc.tensor.matmul(out=ps[:, j:e], lhsT=lhsT,
                                             rhs=rhs_t[:, j:e],
                                             start=not acc, stop=acc)

                    km_ps = midps.tile([128, NBUX], f32, tag="mmp")
                    mm_big(km_ps, wf["Wk"][:], xmean)
                    nc.scalar.activation(out=tbl[:, :NBUX, 0], in_=km_ps[:],
                                         func=ACTF.Identity, scale=SCALE, bias=bk_s[:])
                    nc.scalar.activation(out=tbl[:, :NBUX, 1],
                                         in_=kvbmax[:, :, 0],
                                         func=ACTF.Identity, scale=SCALE, bias=bk_s[:])

                    vm_ps = midps.tile([128, NBUX], f32, tag="mmp")
                    mm_big(vm_ps, wf["Wv"][:], xmean)
                    vmean = midw.tile([128, NBUX], f32, tag="vmean")
                    nc.scalar.activation(out=vmean[:], in_=vm_ps[:],
                                         func=ACTF.Identity, bias=b["bv"][:])
                    vmax = midw.tile([128, NBUX], f32, tag="vmax")
                    nc.scalar.activation(out=vmax[:], in_=kvbmax[:, :, 1],
                                         func=ACTF.Identity, bias=b["bv"][:])
                    vc_ps = midps.tile([128, NBUX], f32, tag="mmp")
                    for j in range(0, NBUX, 512):
                        e = min(j + 512, NBUX)
                        nc.tensor.matmul(out=vc_ps[:, j:e], lhsT=wf["Wvca"][:],
                                         rhs=vmean[:, j:e], start=True, stop=False)
                        nc.tensor.matmul(out=vc_ps[:, j:e], lhsT=wf["Wvcb"][:],
                                         rhs=vmax[:, j:e], start=False, stop=True)
                    nc.scalar.activation(out=tbl[:, :NBUX, 2], in_=vc_ps[:],
                                         func=ACTF.Identity, bias=b["bvc"][:])

                    CTXB = 1024
                    for j in range(0, NSLOT, CTXB):
                        e = min(j + CTXB, NSLOT)
                        nc.gpsimd.ap_gather(
                            out_ap=ctxslot[:, j:e, :], in_ap=tbl[:],
                            idxs_ap=sidx_sb[:, j // 16:e // 16], channels=128,
                            num_elems=NBUX + 16, d=4, num_idxs=e - j)

            # ---------------- pass 2 ----------------
            with tc.tile_pool(name="p2w", bufs=3) as p2w, \
                 tc.tile_pool(name="p2ps", bufs=2, space="PSUM") as p2ps, \
                 tc.tile_pool(name="p2po", bufs=2, space="PSUM") as p2po:
                for cj in range(NCHUNK // 2):
                    xt2 = p2w.tile([128, 2, CHUNK], bf16, tag="xT2")
                    nc.sync.dma_start(
                        out=xt2[:],
                        in_=xtp[:, cj * 2 * CHUNK:(cj + 1) * 2 * CHUNK]
                        .rearrange("p (h k) -> p h k", h=2))
                    oT2 = p2w.tile([128, 2, CHUNK], bf16, tag="oTs")
                    for half in range(2):
                        ci = cj * 2 + half
                        sl0 = ci * 32
                        qT_ps = p2ps.tile([128, CHUNK], f32, tag="qT")
                        nc.tensor.matmul(
                            out=qT_ps[:], lhsT=wb["Wq"][:],
                            rhs=xt2[:, half], start=True, stop=True)
                        ctx = ctxslot[:, sl0:sl0 + 32, :]
                        # qb = q + bq (Act drain PSUM->SBUF bf16)
                        qb = p2w.tile([128, 32, SLOT], bf16, tag="qb")
                        nc.scalar.activation(
                            out=qb[:],
                            in_=qT_ps[:].rearrange("p (s e) -> p s e", e=SLOT),
                            func=ACTF.Identity, bias=b["bq"][:])
                        # inter[u,s,e] = qb[s,e]*ctx[s,u]: DVE (2x) + Pool
                        inter = p2w.tile([128, 2, 32, SLOT], bf16, tag="inter")
                        nc.vector.scalar_tensor_tensor(
                            out=inter[:, 0], in0=qb[:], scalar=0.0,
                            in1=ctx[:, :, 0:1].broadcast_to([128, 32, SLOT]),
                            op0=ALU.add, op1=ALU.mult)
                        nc.gpsimd.tensor_tensor(
                            out=inter[:, 1], in0=qb[:],
                            in1=ctx[:, :, 1:2].broadcast_to([128, 32, SLOT]),
                            op=ALU.mult)
                        h1_ps = p2ps.tile([128, CHUNK], f32, tag="h1")
                        nc.tensor.matmul(
                            out=h1_ps[:], lhsT=wb["Wg1a"][:],
                            rhs=inter[:, 0].rearrange("p a b -> p (a b)"),
                            start=True, stop=False)
                        nc.tensor.matmul(
                            out=h1_ps[:], lhsT=wb["Wg1b"][:],
                            rhs=inter[:, 1].rearrange("p a b -> p (a b)"),
                            start=False, stop=True)
                        # relu on DVE: h1 = max(h1_ps + bg1, 0)
                        h1 = p2w.tile([128, CHUNK], bf16, tag="h1s")
                        nc.vector.tensor_scalar(out=h1[:], in0=h1_ps[:],
                                                scalar1=b["bg1"][:], scalar2=0.0,
                                                op0=ALU.add, op1=ALU.max)
                        h2_ps = p2ps.tile([128, CHUNK], f32, tag="h2")
                        nc.tensor.matmul(out=h2_ps[:], lhsT=wb["Wg2"][:],
                                         rhs=h1[:], start=True, stop=True)
                        gate = p2w.tile([128, 32, SLOT], bf16, tag="gate")
                        nc.scalar.activation(out=gate[:], in_=h2_ps[:],
                                             func=ACTF.Sigmoid, bias=b["bg2"][:])
                        gv = p2w.tile([128, 32, SLOT], bf16, tag="gv")
                        nc.gpsimd.tensor_tensor(
                            out=gv[:], in0=gate[:],
                            in1=ctx[:, :, 2:3].broadcast_to([128, 32, SLOT]),
                            op=ALU.mult)
                        oT_ps = p2po.tile([128, CHUNK], f32, tag="oT")
                        nc.tensor.matmul(
                            out=oT_ps[:], lhsT=wb["Wp"][:],
                            rhs=gv[:].rearrange("p a b -> p (a b)"),
                            start=True, stop=True)
                        # output drain split across DVE / Act
                        nc.vector.tensor_scalar(out=oT2[:, half, :CHUNK // 2],
                                                in0=oT_ps[:, :CHUNK // 2],
                                                scalar1=b["bp"][:], scalar2=None,
                                                op0=ALU.add)
                        nc.scalar.activation(out=oT2[:, half, CHUNK // 2:],
                                             in_=oT_ps[:, CHUNK // 2:],
                                             func=ACTF.Identity, bias=b["bp"][:])
                    nc.gpsimd.dma_start(
                        out=ot[:, cj * 2 * CHUNK:(cj + 1) * 2 * CHUNK],
                        in_=oT2[:].rearrange("p h k -> p (h k)"))

    with tile.TileContext(nc) as tc:
        for _rep in range(reps):
            _emit(tc)
    nc.finalize()
    return nc


def _make_in_maps(inputs, layout):
    shared = {}
    for nm in ["Wq", "Wk", "Wv", "Wg2", "Wp"]:
        shared[nm + "h"] = np.ascontiguousarray(inputs[nm], np.float32).astype(np_bf16)
    wg1 = np.ascontiguousarray(inputs["Wg1"], np.float32)
    shared["Wg1ah"] = wg1[:C].astype(np_bf16)
    shared["Wg1bh"] = wg1[C:].astype(np_bf16)
    del shared["Wg2h"]; shared["Wg2h"] = np.ascontiguousarray(inputs["Wg2"], np.float32).astype(np_bf16)
    for nm in ["Wk", "Wv"]:
        shared[nm + "f"] = np.ascontiguousarray(inputs[nm], np.float32)
    wvc = np.ascontiguousarray(inputs["Wvc"], np.float32)
    shared["Wvcaf"] = wvc[:C]
    shared["Wvcbf"] = wvc[C:]
    for nm in ["bq", "bk", "bv", "bg1", "bg2", "bvc", "bp"]:
        shared[nm] = np.ascontiguousarray(inputs[nm], np.float32)
    x = np.ascontiguousarray(inputs["x"], np.float32)
    in_maps = []
    for core in layout["cores"]:
        perm = core["perm"]
        xp = np.zeros((layout["NPTS"], C), np.float32)
        m = perm >= 0
        xp[m] = x[perm[m]]
        xtp = np.ascontiguousarray(xp.T).astype(np_bf16)
        in_maps.append(dict(shared, xtp=xtp, cnt=core["cnt"], npad=core["npad"],
                            bidx=core["bidx"], sidx=core["sidx"],
                            fidx=core["fidx"]))
    return in_maps


def _assemble_out(results, layout, n):
    out = np.empty((n, C), np.float32)
    for core, r in zip(layout["cores"], results):
        perm = core["perm"]
        m = perm >= 0
        out[perm[m]] = r["ot"].T[m].astype(np.float32)
    return out


def _run(inputs, layout, trace=False):
    nc = _build_nc(layout)
    in_maps = _make_in_maps(inputs, layout)
    res = bass_utils.run_bass_kernel_spmd(
        nc, in_maps, core_ids=list(range(NCORES)), trace=trace)
    out = _assemble_out(res.results, layout, inputs["x"].shape[0])
    return out, res


def kernel(**inputs):
    ids = np.asarray(inputs["cluster_ids"]).astype(np.int64)
    B = int(inputs["total_buckets"])
    layout = _build_layout(ids, B, NCORES)
    out, _ = _run(inputs, layout, trace=False)
    return out


# ---------------------------------------------------------------------------
# pure-numpy emulation of the device program (for logic validation off-HW)
def kernel_emulate(**inputs):
    ids = np.asarray(inputs["cluster_ids"]).astype(np.int64)
    B = int(inputs["total_buckets"])
    L = _build_layout(ids, B, NCORES)
    NPTS, NBUX, NSLOT, SPB = L["NPTS"], L["NBUX"], L["NSLOT"], L["SPB"]
    x = np.asarray(inputs["x"], np.float32)
    W = {k: np.asarray(inputs[k], np.float32) for k in
         ["Wq", "Wk", "Wv", "Wg1", "Wg2", "Wvc", "Wp",
          "bq", "bk", "bv", "bg1", "bg2", "bvc", "bp"]}
    bf = lambda a: a.astype(np_bf16).astype(np.float32)
    Wb = {k: bf(W[k]) for k in ["Wq", "Wk", "Wv", "Wg1", "Wg2", "Wp"]}
    n = x.shape[0]
    out = np.empty((n, C), np.float32)
    for core in L["cores"]:
        perm = core["perm"]
        m = perm >= 0
        xp = np.zeros((NPTS, C), np.float32)
        xp[m] = x[perm[m]]
        xp = bf(xp)  # host bf16 cast
        # pass 1
        kT = bf(xp @ Wb["Wk"]).T
        vT = bf(xp @ Wb["Wv"]).T
        xslot = np.zeros((128, NSLOT + 16), np.float32)
        kslot = np.full((128, NSLOT + 16), NEG_BIG, np.float32)
        vslot = np.full((128, NSLOT + 16), NEG_BIG, np.float32)
        xt = xp.T.reshape(128, NSLOT, SLOT)
        t = bf(xt[:, :, 0::2] + xt[:, :, 1::2])
        t = bf(t[:, :, 0::2] + t[:, :, 1::2])
        t = bf(t[:, :, 0::2] + t[:, :, 1::2])
        xslot[:, :NSLOT] = t[:, :, 0] + t[:, :, 1]
        kslot[:, :NSLOT] = bf(kT.reshape(128, NSLOT, SLOT).max(axis=2))
        vslot[:, :NSLOT] = bf(vT.reshape(128, NSLOT, SLOT).max(axis=2))
        firsts = np.zeros((128, NSLOT + 16), np.float32)
        firsts[:, :NSLOT] = xp.T.reshape(128, NSLOT, SLOT)[:, :, 0]
        # mid
        def unwrap(arr, nn):
            outv = np.zeros(nn, np.int64)
            k = np.arange(nn)
            outv[k] = arr[(k % 16), k // 16]
            return outv
        bidx = unwrap(core["bidx"], NBUX * SPB)
        sidx = unwrap(core["sidx"], NSLOT)
        fidx = unwrap(core["fidx"], NBUX)
        xbsum = xslot[:, bidx].reshape(128, NBUX, SPB).sum(axis=2)
        kbmax = kslot[:, bidx].reshape(128, NBUX, SPB).max(axis=2)
        vbmax = vslot[:, bidx].reshape(128, NBUX, SPB).max(axis=2)
        fb = firsts[:, fidx]
        xsum = xbsum - core["npad"] * fb
        rc = 1.0 / core["cnt"]
        xmean = xsum * rc
        tbl = np.zeros((128, NBUX + 16, 4), np.float32)
        tbl[:, :NBUX, 0] = bf(SCALE * (W["Wk"].T @ xmean + W["bk"][:, None]))
        tbl[:, :NBUX, 1] = bf(SCALE * (kbmax + W["bk"][:, None]))
        vmean = W["Wv"].T @ xmean + W["bv"][:, None]
        vmax = vbmax + W["bv"][:, None]
        tbl[:, :NBUX, 2] = bf(W["Wvc"][:C].T @ vmean + W["Wvc"][C:].T @ vmax
                              + W["bvc"][:, None])
        ctxslot = tbl[:, sidx, :]  # [128, NSLOT, 4]
        # pass 2
        qT = bf((xp @ Wb["Wq"]).T + W["bq"][:, None])
        ctxe = np.repeat(ctxslot, SLOT, axis=1)  # [128, NPTS, 4]
        inter1 = bf(qT * ctxe[:, :, 0])
        inter2 = bf(qT * ctxe[:, :, 1])
        h1 = bf(np.maximum(Wb["Wg1"][:C].T @ inter1 + Wb["Wg1"][C:].T @ inter2
                           + W["bg1"][:, None], 0.0))
        h2 = Wb["Wg2"].T @ h1 + W["bg2"][:, None]
        gate = bf(1.0 / (1.0 + np.exp(-h2)))
        gv = bf(gate * ctxe[:, :, 2])
        oT = bf(Wb["Wp"].T @ gv + W["bp"][:, None])
        out[perm[m]] = oT.T[m]
    return out
